# revision 1
# baseline (speedup 1.0000x reference)
"""Bass/Trainium2 kernel for nn_DescriptorNetwork (gnn_message_passing).

Strategy (8 NeuronCores, SPMD):
- Host: sort edges by self node; partition crystals (6250/core) -> contiguous
  node/edge ranges per core; window-pad edges (128-node aligned windows,
  uniform edges/window); fold gate-MLP w2 into w1 columns (|c| scale +
  sign-split) so the per-edge gate scalar comes out of ScalarE accum_out;
  host computes the input embedding x0 (tiny fraction of FLOPs).
- Device per layer: per 128-edge chunk: gather x_nbr rows (indirect DMA),
  build selection matrix S from precomputed shifts (DVE is_equal), expand
  x_self^T via one-hot matmul with S^T, run W1 as fea^T-stationary matmul,
  gate scalar via Prelu+accum_out (sign-split), q = exp(gate + p*ln(w)+b2g),
  q-scaled leaky msg hidden (scale fused in ACT), segment-sum via selection
  matmul into PSUM window accumulators (z-commute: msg W2 applied per node
  window after the reduction), flush windows: normalize by 1/(den+eps),
  transpose z, W2m + bias matmuls, residual add, write x_{l+1}.
- AllGather x slices between layers; crystal pooling = same machinery on
  nodes->crystals with local gathers only.
"""
import sys
import os

sys.path.insert(0, '/opt/trn_rl_repo')
sys.path.insert(0, os.path.dirname(os.path.abspath(__file__)))

import numpy as np
import ml_dtypes

N_NODES = 250_000
N_EDGES = 1_250_000
N_CRY = 50_000
EMB = 200
FEA = 64
HID = 256
NG_L = 3
EH = 3
CH = 3
NCORES = 8
P = 128
EPS = 1e-10

_cache = {}


def _wrapped_none():
    pass


def _prep(inputs):
    f32 = np.float32
    ew = np.asarray(inputs["elem_weights"], f32)          # (N,1)
    ef = np.asarray(inputs["elem_fea"], f32)              # (N,200)
    sidx = np.asarray(inputs["self_fea_idx"], np.int64)
    nidx = np.asarray(inputs["nbr_fea_idx"], np.int64)
    cry = np.asarray(inputs["cry_elem_idx"], np.int64)

    emb_w = np.asarray(inputs["emb_w"], f32)
    emb_b = np.asarray(inputs["emb_b"], f32)

    # host embedding -> x0 (N, 64)
    x0 = np.concatenate([ef @ emb_w + emb_b, ew], axis=1).astype(f32)

    # sort edges by self
    order = np.argsort(sidx, kind="stable")
    s_s = sidx[order]
    s_n = nidx[order]
    lnw_n = np.log(ew[:, 0])[s_n]                          # ln w_nbr per sorted edge

    # core partition by crystals
    cpc = N_CRY // NCORES
    node_start = np.searchsorted(cry, np.arange(0, N_CRY + 1, cpc))
    edge_start = np.searchsorted(s_s, node_start)
    NKs = np.diff(node_start)
    NW = int(np.ceil(NKs.max() / 128)) + 1                 # windows per core
    SLOT = NW * 128

    # per (core, window) edge counts -> uniform WE
    WE = 0
    per_core = []
    for k in range(NCORES):
        e0, e1 = edge_start[k], edge_start[k + 1]
        sl = (s_s[e0:e1] - node_start[k]).astype(np.int64)
        win = sl >> 7
        cnt = np.bincount(win, minlength=NW)
        WE = max(WE, int(cnt.max()))
        per_core.append((sl, s_n[e0:e1], lnw_n[e0:e1], cnt))
    WE = ((WE + 127) // 128) * 128
    WE_CH = WE // 128
    EP = NW * WE
    NCHUNK = EP // 128

    g_pow = np.asarray(inputs["g_pow"], f32)
    gb2 = np.asarray(inputs["g_gate_b2"], f32)             # (3,3,1)
    for nm in ("g_gate_b1", "g_msg_b1", "c_gate_b1", "c_msg_b1"):
        assert not np.any(np.asarray(inputs[nm])), f"{nm} nonzero unsupported"

    # ---- per-core edge arrays ----
    cores = []
    for k in range(NCORES):
        sl, nl, lnw, cnt = per_core[k]
        win = sl >> 7
        src_base = np.concatenate([[0], np.cumsum(cnt)])
        pos_in_win = np.arange(len(sl)) - src_base[win]
        dst = win * WE + pos_in_win
        owner = np.searchsorted(node_start, nl, side="right") - 1
        npos = owner * SLOT + (nl - node_start[owner])

        gnbr = np.zeros(EP, np.int32)
        shift = np.full(EP, -1.0, f32)
        plnw = np.full((NG_L, EP, EH), -60.0, f32)
        gnbr[dst] = npos
        shift[dst] = (sl - (win << 7)).astype(f32)
        for l in range(NG_L):
            for h in range(EH):
                plnw[l, dst, h] = g_pow[l, h] * lnw + gb2[l, h, 0]

        gnbr_pm = np.ascontiguousarray(gnbr.reshape(NCHUNK, 128).T)
        shift_pm = np.ascontiguousarray(shift.reshape(NCHUNK, 128).T)
        plnw_pm = np.ascontiguousarray(
            plnw.reshape(NG_L, NCHUNK, 128, EH).transpose(0, 2, 1, 3))
        cores.append(dict(gnbr=gnbr_pm, shift=shift_pm, plnw=plnw_pm))

    # ---- xfull0 / xloc0 ----
    xfull0 = np.zeros((NCORES * SLOT, 64), f32)
    for k in range(NCORES):
        n0, n1 = node_start[k], node_start[k + 1]
        xfull0[k * SLOT:k * SLOT + (n1 - n0)] = x0[n0:n1]
    for k in range(NCORES):
        n0, n1 = node_start[k], node_start[k + 1]
        cores[k]["xloc0"] = np.ascontiguousarray(
            xfull0[k * SLOT:(k + 1) * SLOT])

    # ---- crystal stage ----
    c_pow = np.asarray(inputs["c_pow"], f32)
    cb2 = np.asarray(inputs["c_gate_b2"], f32)             # (3,1)
    NWC = int(np.ceil(cpc / 128)) + 1
    WNC = 0
    ccore = []
    for k in range(NCORES):
        n0, n1 = node_start[k], node_start[k + 1]
        cl = (cry[n0:n1] - k * cpc).astype(np.int64)
        cwin = cl >> 7
        ccnt = np.bincount(cwin, minlength=NWC)
        WNC = max(WNC, int(ccnt.max()))
        ccore.append((cl, ccnt, n0, n1))
    WNC = ((WNC + 127) // 128) * 128
    WNC_CH = WNC // 128
    SP = NWC * WNC
    NCC = SP // 128
    for k in range(NCORES):
        cl, ccnt, n0, n1 = ccore[k]
        cwin = cl >> 7
        src_base = np.concatenate([[0], np.cumsum(ccnt)])
        piw = np.arange(len(cl)) - src_base[cwin]
        dst = cwin * WNC + piw
        cidx = np.zeros(SP, np.int32)
        cshift = np.full(SP, -1.0, f32)
        cplnw = np.full((SP, CH), -60.0, f32)
        cidx[dst] = np.arange(n1 - n0, dtype=np.int32)     # local node rows
        cshift[dst] = (cl - (cwin << 7)).astype(f32)
        lnwl = np.log(ew[n0:n1, 0])
        for h in range(CH):
            cplnw[dst, h] = c_pow[h] * lnwl + cb2[h, 0]
        cores[k]["cidx"] = np.ascontiguousarray(cidx.reshape(NCC, 128).T)
        cores[k]["cshift"] = np.ascontiguousarray(cshift.reshape(NCC, 128).T)
        cores[k]["cplnw"] = np.ascontiguousarray(
            cplnw.reshape(NCC, 128, CH).transpose(1, 0, 2))

    # ---- weights fold ----
    bf16 = ml_dtypes.bfloat16

    def fold(gw1, gw2, mw1):
        """gw1 (din,256), gw2 (256,1), mw1 (din,256) ->
        W1 (din, 1536-ish) cols [gate-folded sign-permuted | msg], kpos"""
        c = gw2[:, 0]
        order = np.argsort(c <= 0, kind="stable")
        kpos = int((c > 0).sum())
        gfold = gw1[:, order] * np.abs(c[order])[None, :]
        w1c = np.concatenate([gfold, mw1], axis=1)
        if w1c.shape[0] == 128:
            w1c = np.concatenate([w1c[64:], w1c[:64]], axis=0)  # [nbr; self]
        return w1c, kpos

    W1L = np.zeros((NG_L, 128, 2 * EH * HID), f32)
    kposL = np.zeros((NG_L, EH), np.int64)
    W2M = np.zeros((NG_L, EH, 2, 128, 64), f32)
    B2M = np.zeros((NG_L, EH, 64), f32)
    gg1 = np.asarray(inputs["g_gate_w1"], f32)
    gg2 = np.asarray(inputs["g_gate_w2"], f32)
    gm1 = np.asarray(inputs["g_msg_w1"], f32)
    gm2 = np.asarray(inputs["g_msg_w2"], f32)
    gmb2 = np.asarray(inputs["g_msg_b2"], f32)
    for l in range(NG_L):
        for h in range(EH):
            w1c, kp = fold(gg1[l, h], gg2[l, h], gm1[l, h])
            kposL[l, h] = kp
            W1L[l, :, h * HID:(h + 1) * HID] = w1c[:, :HID]
            W1L[l, :, (EH + h) * HID:(EH + h + 1) * HID] = w1c[:, HID:]
            W2M[l, h, 0] = gm2[l, h][:128] / EH
            W2M[l, h, 1] = gm2[l, h][128:] / EH
            B2M[l, h] = gmb2[l, h] / EH
    cg1 = np.asarray(inputs["c_gate_w1"], f32)
    cg2 = np.asarray(inputs["c_gate_w2"], f32)
    cm1 = np.asarray(inputs["c_msg_w1"], f32)
    cm2 = np.asarray(inputs["c_msg_w2"], f32)
    cmb2 = np.asarray(inputs["c_msg_b2"], f32)
    W1C = np.zeros((64, 2 * CH * HID), f32)
    kposC = np.zeros(CH, np.int64)
    W2MC = np.zeros((CH, 2, 128, 64), f32)
    B2MC = np.zeros((CH, 64), f32)
    for h in range(CH):
        w1c, kp = fold(cg1[h], cg2[h], cm1[h])
        kposC[h] = kp
        W1C[:, h * HID:(h + 1) * HID] = w1c[:, :HID]
        W1C[:, (CH + h) * HID:(CH + h + 1) * HID] = w1c[:, HID:]
        W2MC[h, 0] = cm2[h][:128] / CH
        W2MC[h, 1] = cm2[h][128:] / CH
        B2MC[h] = cmb2[h] / CH

    shared = dict(
        W1L=W1L.astype(bf16), W2M=W2M.astype(bf16), B2M=B2M.astype(bf16),
        W1C=W1C.astype(bf16), W2MC=W2MC.astype(bf16), B2MC=B2MC.astype(bf16),
        xfull0=xfull0)
    dims = dict(NW=NW, SLOT=SLOT, WE=WE, WE_CH=WE_CH, EP=EP, NCHUNK=NCHUNK,
                NWC=NWC, WNC=WNC, WNC_CH=WNC_CH, SP=SP, NCC=NCC,
                kposL=tuple(map(tuple, kposL.tolist())),
                kposC=tuple(kposC.tolist()), cpc=cpc)
    return cores, shared, dims, node_start


def _build(dims):
    import concourse.bass as bass
    import concourse.bacc as bacc
    import concourse.mybir as mybir
    from concourse.tile import TileContext
    from concourse.masks import make_identity

    F32 = mybir.dt.float32
    BF16 = mybir.dt.bfloat16
    I32 = mybir.dt.int32
    AF = mybir.ActivationFunctionType
    OP = mybir.AluOpType

    NW, SLOT, WE_CH, NCHUNK = dims["NW"], dims["SLOT"], dims["WE_CH"], dims["NCHUNK"]
    NWC, WNC_CH, NCC = dims["NWC"], dims["WNC_CH"], dims["NCC"]
    kposL, kposC = dims["kposL"], dims["kposC"]
    HW = 2 * EH * HID     # 1536

    nc = bacc.Bacc("TRN2", target_bir_lowering=False, debug=False,
                   num_devices=NCORES)
    T = {}
    T["xfull0"] = nc.dram_tensor("xfull0", [NCORES * SLOT, 64], F32, kind="ExternalInput")
    T["xloc0"] = nc.dram_tensor("xloc0", [SLOT, 64], F32, kind="ExternalInput")
    T["gnbr"] = nc.dram_tensor("gnbr", [128, NCHUNK], I32, kind="ExternalInput")
    T["shift"] = nc.dram_tensor("shift", [128, NCHUNK], F32, kind="ExternalInput")
    T["plnw"] = nc.dram_tensor("plnw", [NG_L, 128, NCHUNK, EH], F32, kind="ExternalInput")
    T["W1L"] = nc.dram_tensor("W1L", [NG_L, 128, HW], BF16, kind="ExternalInput")
    T["W2M"] = nc.dram_tensor("W2M", [NG_L, EH, 2, 128, 64], BF16, kind="ExternalInput")
    T["B2M"] = nc.dram_tensor("B2M", [NG_L, EH, 64], BF16, kind="ExternalInput")
    T["cidx"] = nc.dram_tensor("cidx", [128, NCC], I32, kind="ExternalInput")
    T["cshift"] = nc.dram_tensor("cshift", [128, NCC], F32, kind="ExternalInput")
    T["cplnw"] = nc.dram_tensor("cplnw", [128, NCC, CH], F32, kind="ExternalInput")
    T["W1C"] = nc.dram_tensor("W1C", [64, 2 * CH * HID], BF16, kind="ExternalInput")
    T["W2MC"] = nc.dram_tensor("W2MC", [CH, 2, 128, 64], BF16, kind="ExternalInput")
    T["B2MC"] = nc.dram_tensor("B2MC", [CH, 64], BF16, kind="ExternalInput")
    out_d = nc.dram_tensor("out", [NWC * 128, 64], F32, kind="ExternalOutput")

    with TileContext(nc) as tc:
        with tc.tile_pool(name="const", bufs=1) as cst, \
             tc.tile_pool(name="dram", bufs=1, space="DRAM") as dpool, \
             tc.tile_pool(name="work", bufs=3) as wk, \
             tc.tile_pool(name="flush", bufs=2) as fl, \
             tc.tile_pool(name="psA", bufs=2, space="PSUM") as psA, \
             tc.tile_pool(name="psB", bufs=1, space="PSUM") as psB, \
             tc.tile_pool(name="psC", bufs=1, space="PSUM") as psC:

            ident = cst.tile([128, 128], F32)
            make_identity(nc, ident[:])
            ident_bf = cst.tile([128, 128], BF16)
            nc.vector.tensor_copy(out=ident_bf[:], in_=ident[:])
            iota = cst.tile([128, 128], F32)
            nc.gpsimd.iota(iota[:], pattern=[[1, 128]], base=0,
                           channel_multiplier=0,
                           allow_small_or_imprecise_dtypes=True)
            # resident weights
            W1sb = cst.tile([128, NG_L, HW], BF16)
            nc.sync.dma_start(out=W1sb[:],
                              in_=T["W1L"][:, :, :].rearrange("l p w -> p l w"))
            W2sb = cst.tile([128, NG_L, EH, 2, 64], BF16)
            nc.sync.dma_start(out=W2sb[:],
                              in_=T["W2M"][:, :, :, :, :].rearrange("l h k p d -> p l h k d"))
            B2sb = cst.tile([EH, NG_L, 64], BF16)
            nc.sync.dma_start(out=B2sb[:],
                              in_=T["B2M"][:, :, :].rearrange("l h d -> h l d"))
            W1Csb = cst.tile([64, 2 * CH * HID], BF16)
            nc.sync.dma_start(out=W1Csb[:], in_=T["W1C"][:, :])
            W2Csb = cst.tile([128, CH, 2, 64], BF16)
            nc.sync.dma_start(out=W2Csb[:],
                              in_=T["W2MC"][:, :, :, :].rearrange("h k p d -> p h k d"))
            B2Csb = cst.tile([CH, 64], BF16)
            nc.sync.dma_start(out=B2Csb[:], in_=T["B2MC"][:, :])
            gnbr_sb = cst.tile([128, NCHUNK], I32)
            nc.sync.dma_start(out=gnbr_sb[:], in_=T["gnbr"][:, :])
            shift_sb = cst.tile([128, NCHUNK], F32)
            nc.sync.dma_start(out=shift_sb[:], in_=T["shift"][:, :])
            cidx_sb = cst.tile([128, NCC], I32)
            nc.sync.dma_start(out=cidx_sb[:], in_=T["cidx"][:, :])
            cshift_sb = cst.tile([128, NCC], F32)
            nc.sync.dma_start(out=cshift_sb[:], in_=T["cshift"][:, :])
            cplnw_sb = cst.tile([128, NCC, CH], F32)
            nc.sync.dma_start(out=cplnw_sb[:], in_=T["cplnw"][:, :, :])

            # dram intermediates
            xloc = [None] * 4
            xloc[0] = T["xloc0"]
            for i in (1, 2, 3):
                t = dpool.tile([SLOT, 64], F32, name=f"xloc{i}", tag=f"xl{i}")
                xloc[i] = t
            agin = [dpool.tile([SLOT, 64], F32, name=f"agin{i}", tag=f"ag{i}")
                    for i in range(2)]
            xfull = [None, None, None]
            xfull[0] = T["xfull0"]
            for i in (1, 2):
                xfull[i] = dpool.tile([NCORES * SLOT, 64], F32,
                                      name=f"xfull{i}", tag=f"xf{i}")

            plnw_sb = cst.tile([128, NG_L, NCHUNK, EH], F32)
            nc.sync.dma_start(out=plnw_sb[:],
                              in_=T["plnw"][:, :, :, :].rearrange("l p c h -> p l c h"))

            def xloc_ap(i):
                return xloc[i][:, :] if i == 0 else xloc[i][:]

            def xfull_ap(i):
                return xfull[i][:, :] if i == 0 else xfull[i][:]

            # ---------------- graph layers ----------------
            def graph_layer(l):
                kp = kposL[l]
                for w in range(NW):
                    xb = wk.tile([128, 64], F32, tag="xb", name="xb")
                    nc.sync.dma_start(out=xb[:],
                                      in_=xloc_ap(l)[w * 128:(w + 1) * 128, :])
                    xbb = wk.tile([128, 64], BF16, tag="xbb", name="xbb")
                    nc.vector.tensor_copy(out=xbb[:], in_=xb[:])
                    acc = psC.tile([128, 772], F32, tag="acc", name="acc")
                    for cw in range(WE_CH):
                        c = w * WE_CH + cw
                        # selection matrix from shifts
                        S = wk.tile([128, 128], BF16, tag="S", name="S")
                        nc.vector.tensor_scalar(
                            out=S[:], in0=iota[:], scalar1=shift_sb[:, c:c + 1],
                            scalar2=None, op0=OP.is_equal)
                        # S^T for self expansion
                        pST = psA.tile([128, 128], BF16, tag="pA", name="pST")
                        nc.tensor.transpose(out=pST[:], in_=S[:], identity=ident_bf[:])
                        ST = wk.tile([128, 128], BF16, tag="ST", name="ST")
                        nc.scalar.activation(out=ST[:], in_=pST[:], func=AF.Copy)
                        # nbr gather + transpose
                        gt = wk.tile([128, 64], F32, tag="gt", name="gt")
                        nc.gpsimd.indirect_dma_start(
                            out=gt[:], out_offset=None, in_=xfull_ap(l),
                            in_offset=bass.IndirectOffsetOnAxis(
                                ap=gnbr_sb[:, c:c + 1], axis=0))
                        pF = psA.tile([128, 128], F32, tag="pA", name="pF")
                        nc.tensor.transpose(out=pF[0:64, :], in_=gt[:],
                                            identity=ident[:])
                        nc.tensor.matmul(out=pF[64:128, :], lhsT=xbb[:], rhs=ST[:],
                                         start=True, stop=True)
                        feaT = wk.tile([128, 128], BF16, tag="feaT", name="feaT")
                        nc.vector.tensor_copy(out=feaT[:], in_=pF[:])
                        # W1
                        h1 = psB.tile([128, HW], F32, tag="h1", name="h1")
                        for j in range(HW // 512):
                            nc.tensor.matmul(out=h1[:, j * 512:(j + 1) * 512],
                                             lhsT=feaT[:],
                                             rhs=W1sb[:, l, j * 512:(j + 1) * 512],
                                             start=True, stop=True)
                        # gate accums (sign-split leaky sums)
                        gsc = wk.tile([128, 8], F32, tag="gsc", name="gsc")
                        junk = wk.tile([128, 256], BF16, tag="junk", name="junk")
                        for h in range(EH):
                            k0 = kp[h]
                            base = h * HID
                            if k0 > 0:
                                nc.scalar.activation(
                                    out=junk[:, :k0], in_=h1[:, base:base + k0],
                                    func=AF.Prelu, alpha=0.01,
                                    accum_out=gsc[:, h:h + 1])
                            else:
                                nc.vector.memset(gsc[:, h:h + 1], 0.0)
                            if k0 < HID:
                                nc.scalar.activation(
                                    out=junk[:, :HID - k0],
                                    in_=h1[:, base + k0:base + HID],
                                    func=AF.Prelu, alpha=0.01,
                                    accum_out=gsc[:, 3 + h:4 + h])
                            else:
                                nc.vector.memset(gsc[:, 3 + h:4 + h], 0.0)
                        # q = exp(gpos - gneg + plnw)
                        q3 = wk.tile([128, 3], F32, tag="q3", name="q3")
                        nc.vector.tensor_tensor(out=q3[:], in0=gsc[:, 0:3],
                                                in1=gsc[:, 3:6], op=OP.subtract)
                        nc.vector.tensor_tensor(out=q3[:], in0=q3[:],
                                                in1=plnw_sb[:, l, c, :], op=OP.add)
                        nc.scalar.activation(out=q3[:], in_=q3[:], func=AF.Exp)
                        # msg hidden: q-scaled leaky -> sbuf
                        mq = wk.tile([128, 772], BF16, tag="mq", name="mq")
                        for h in range(EH):
                            nc.scalar.activation(
                                out=mq[:, h * HID:(h + 1) * HID],
                                in_=h1[:, (EH + h) * HID:(EH + h + 1) * HID],
                                func=AF.Prelu, alpha=0.01, scale=q3[:, h:h + 1])
                        nc.vector.tensor_copy(out=mq[:, 768:771], in_=q3[:])
                        # selection matmul into window accumulator
                        nc.tensor.matmul(out=acc[:, 0:512], lhsT=S[:],
                                         rhs=mq[:, 0:512], start=(cw == 0),
                                         stop=(cw == WE_CH - 1))
                        nc.tensor.matmul(out=acc[:, 512:771], lhsT=S[:],
                                         rhs=mq[:, 512:771], start=(cw == 0),
                                         stop=(cw == WE_CH - 1))
                    # ---- flush window w ----
                    rec = fl.tile([128, 3], F32, tag="rec", name="rec")
                    nc.vector.tensor_scalar(out=rec[:], in0=acc[:, 768:771],
                                            scalar1=EPS, scalar2=None, op0=OP.add)
                    nc.vector.reciprocal(out=rec[:], in_=rec[:])
                    z = fl.tile([128, 896], BF16, tag="z", name="z")
                    for h in range(EH):
                        nc.vector.tensor_scalar(
                            out=z[:, h * HID:(h + 1) * HID],
                            in0=acc[:, h * HID:(h + 1) * HID],
                            scalar1=rec[:, h:h + 1], scalar2=None, op0=OP.mult)
                    nc.vector.tensor_tensor(out=z[:, 768:771], in0=acc[:, 768:771],
                                            in1=rec[:], op=OP.mult)
                    po = psA.tile([64, 128], F32, tag="pA", name="po")
                    for h in range(EH):
                        for kk in range(2):
                            pzT = psA.tile([128, 128], BF16, tag="pA", name="pzT")
                            nc.tensor.transpose(
                                out=pzT[:],
                                in_=z[:, (2 * h + kk) * 128:(2 * h + kk + 1) * 128],
                                identity=ident_bf[:])
                            zT = fl.tile([128, 128], BF16, tag="zT",
                                         name=f"zT{h}{kk}")
                            nc.scalar.activation(out=zT[:], in_=pzT[:], func=AF.Copy)
                            nc.tensor.matmul(out=po[:], lhsT=W2sb[:, l, h, kk, :],
                                             rhs=zT[:], start=(h == 0 and kk == 0),
                                             stop=False)
                    pdT = psA.tile([3, 128], BF16, tag="pA", name="pdT")
                    nc.tensor.transpose(out=pdT[:], in_=z[:, 768:771],
                                        identity=ident_bf[:])
                    dT = fl.tile([3, 128], BF16, tag="dT", name="dT")
                    nc.scalar.activation(out=dT[:], in_=pdT[:], func=AF.Copy)
                    nc.tensor.matmul(out=po[:], lhsT=B2sb[:, l, :], rhs=dT[:],
                                     start=False, stop=True)
                    oT = fl.tile([64, 128], F32, tag="oT", name="oT")
                    nc.vector.tensor_copy(out=oT[:], in_=po[:])
                    px = psA.tile([128, 64], F32, tag="pA", name="px")
                    nc.tensor.transpose(out=px[:], in_=oT[:], identity=ident[0:64, 0:64])
                    xn = fl.tile([128, 64], F32, tag="xn", name="xn")
                    nc.vector.tensor_tensor(out=xn[:], in0=px[:], in1=xb[:],
                                            op=OP.add)
                    nc.sync.dma_start(
                        out=xloc_ap(l + 1)[w * 128:(w + 1) * 128, :], in_=xn[:])
                    if l < 2:
                        nc.sync.dma_start(
                            out=agin[l][:][w * 128:(w + 1) * 128, :], in_=xn[:])
                if l < 2:
                    nc.gpsimd.collective_compute(
                        "AllGather", mybir.AluOpType.bypass,
                        replica_groups=[list(range(NCORES))],
                        ins=[agin[l].opt()], outs=[xfull[l + 1].opt()])

            for l in range(NG_L):
                graph_layer(l)

            # ---------------- crystal pooling ----------------
            HWC = 2 * CH * HID
            for w in range(NWC):
                acc = psC.tile([128, 772], F32, tag="acc", name="cacc")
                for cw in range(WNC_CH):
                    c = w * WNC_CH + cw
                    S = wk.tile([128, 128], BF16, tag="S", name="cS")
                    nc.vector.tensor_scalar(
                        out=S[:], in0=iota[:], scalar1=cshift_sb[:, c:c + 1],
                        scalar2=None, op0=OP.is_equal)
                    gt = wk.tile([128, 64], F32, tag="gt", name="cgt")
                    nc.gpsimd.indirect_dma_start(
                        out=gt[:], out_offset=None, in_=xloc[3][:],
                        in_offset=bass.IndirectOffsetOnAxis(
                            ap=cidx_sb[:, c:c + 1], axis=0))
                    pF = psA.tile([64, 128], F32, tag="pA", name="cpF")
                    nc.tensor.transpose(out=pF[:], in_=gt[:], identity=ident[:])
                    feaT = wk.tile([64, 128], BF16, tag="cfeaT", name="cfeaT")
                    nc.vector.tensor_copy(out=feaT[:], in_=pF[:])
                    h1 = psB.tile([128, HWC], F32, tag="h1", name="ch1")
                    for j in range(HWC // 512):
                        nc.tensor.matmul(out=h1[:, j * 512:(j + 1) * 512],
                                         lhsT=feaT[:],
                                         rhs=W1Csb[:, j * 512:(j + 1) * 512],
                                         start=True, stop=True)
                    gsc = wk.tile([128, 8], F32, tag="gsc", name="cgsc")
                    junk = wk.tile([128, 256], BF16, tag="junk", name="cjunk")
                    for h in range(CH):
                        k0 = kposC[h]
                        base = h * HID
                        if k0 > 0:
                            nc.scalar.activation(
                                out=junk[:, :k0], in_=h1[:, base:base + k0],
                                func=AF.Prelu, alpha=0.01,
                                accum_out=gsc[:, h:h + 1])
                        else:
                            nc.vector.memset(gsc[:, h:h + 1], 0.0)
                        if k0 < HID:
                            nc.scalar.activation(
                                out=junk[:, :HID - k0],
                                in_=h1[:, base + k0:base + HID],
                                func=AF.Prelu, alpha=0.01,
                                accum_out=gsc[:, 3 + h:4 + h])
                        else:
                            nc.vector.memset(gsc[:, 3 + h:4 + h], 0.0)
                    q3 = wk.tile([128, 3], F32, tag="q3", name="cq3")
                    nc.vector.tensor_tensor(out=q3[:], in0=gsc[:, 0:3],
                                            in1=gsc[:, 3:6], op=OP.subtract)
                    nc.vector.tensor_tensor(out=q3[:], in0=q3[:],
                                            in1=cplnw_sb[:, c, :], op=OP.add)
                    nc.scalar.activation(out=q3[:], in_=q3[:], func=AF.Exp)
                    mq = wk.tile([128, 772], BF16, tag="mq", name="cmq")
                    for h in range(CH):
                        nc.scalar.activation(
                            out=mq[:, h * HID:(h + 1) * HID],
                            in_=h1[:, (CH + h) * HID:(CH + h + 1) * HID],
                            func=AF.Prelu, alpha=0.01, scale=q3[:, h:h + 1])
                    nc.vector.tensor_copy(out=mq[:, 768:771], in_=q3[:])
                    nc.tensor.matmul(out=acc[:, 0:512], lhsT=S[:],
                                     rhs=mq[:, 0:512], start=(cw == 0),
                                     stop=(cw == WNC_CH - 1))
                    nc.tensor.matmul(out=acc[:, 512:771], lhsT=S[:],
                                     rhs=mq[:, 512:771], start=(cw == 0),
                                     stop=(cw == WNC_CH - 1))
                rec = fl.tile([128, 3], F32, tag="rec", name="crec")
                nc.vector.tensor_scalar(out=rec[:], in0=acc[:, 768:771],
                                        scalar1=EPS, scalar2=None, op0=OP.add)
                nc.vector.reciprocal(out=rec[:], in_=rec[:])
                z = fl.tile([128, 896], BF16, tag="z", name="cz")
                for h in range(CH):
                    nc.vector.tensor_scalar(
                        out=z[:, h * HID:(h + 1) * HID],
                        in0=acc[:, h * HID:(h + 1) * HID],
                        scalar1=rec[:, h:h + 1], scalar2=None, op0=OP.mult)
                nc.vector.tensor_tensor(out=z[:, 768:771], in0=acc[:, 768:771],
                                        in1=rec[:], op=OP.mult)
                po = psA.tile([64, 128], F32, tag="pA", name="cpo")
                for h in range(CH):
                    for kk in range(2):
                        pzT = psA.tile([128, 128], BF16, tag="pA", name="cpzT")
                        nc.tensor.transpose(
                            out=pzT[:],
                            in_=z[:, (2 * h + kk) * 128:(2 * h + kk + 1) * 128],
                            identity=ident_bf[:])
                        zT = fl.tile([128, 128], BF16, tag="zT", name=f"czT{h}{kk}")
                        nc.scalar.activation(out=zT[:], in_=pzT[:], func=AF.Copy)
                        nc.tensor.matmul(out=po[:], lhsT=W2Csb[:, h, kk, :], rhs=zT[:],
                                         start=(h == 0 and kk == 0), stop=False)
                pdT = psA.tile([3, 128], BF16, tag="pA", name="cpdT")
                nc.tensor.transpose(out=pdT[:], in_=z[:, 768:771], identity=ident_bf[:])
                dT = fl.tile([3, 128], BF16, tag="dT", name="cdT")
                nc.scalar.activation(out=dT[:], in_=pdT[:], func=AF.Copy)
                nc.tensor.matmul(out=po[:], lhsT=B2Csb[:], rhs=dT[:],
                                 start=False, stop=True)
                oT = fl.tile([64, 128], F32, tag="oT", name="coT")
                nc.vector.tensor_copy(out=oT[:], in_=po[:])
                px = psA.tile([128, 64], F32, tag="pA", name="cpx")
                nc.tensor.transpose(out=px[:], in_=oT[:], identity=ident[0:64, 0:64])
                xn = fl.tile([128, 64], F32, tag="xn", name="cxn")
                nc.vector.tensor_copy(out=xn[:], in_=px[:])
                nc.sync.dma_start(out=out_d[w * 128:(w + 1) * 128, :], in_=xn[:])

    nc.compile()
    return nc


# ---------------- inlined PJRT runner ----------------
import time as _time


def make_runner(nc, n_cores):
    import jax
    from jax.sharding import Mesh, PartitionSpec
    from jax.experimental.shard_map import shard_map
    import concourse.mybir as mybir
    from concourse.bass2jax import _bass_exec_p, install_neuronx_cc_hook, partition_id_tensor

    install_neuronx_cc_hook()
    partition_name = nc.partition_id_tensor.name if nc.partition_id_tensor else None
    in_names, out_names, out_avals, zero_outs = [], [], [], []
    for alloc in nc.m.functions[0].allocations:
        if not isinstance(alloc, mybir.MemoryLocationSet):
            continue
        name = alloc.memorylocations[0].name
        if alloc.kind == "ExternalInput":
            if name != partition_name:
                in_names.append(name)
        elif alloc.kind == "ExternalOutput":
            shape = tuple(alloc.tensor_shape)
            dtype = mybir.dt.np(alloc.dtype)
            out_names.append(name)
            out_avals.append(jax.core.ShapedArray(shape, dtype))
            zero_outs.append(np.zeros(shape, dtype))
    n_params = len(in_names)
    n_outs = len(out_avals)
    all_in_names = list(in_names) + list(out_names)
    if partition_name is not None:
        all_in_names.append(partition_name)

    def _body(*args):
        operands = list(args)
        if partition_name is not None:
            operands.append(partition_id_tensor())
        outs = _bass_exec_p.bind(
            *operands,
            out_avals=tuple(out_avals),
            in_names=tuple(all_in_names),
            out_names=tuple(out_names),
            lowering_input_output_aliases=(),
            sim_require_finite=False,
            sim_require_nnan=False,
            nc=nc,
        )
        return tuple(outs)

    donate = tuple(range(n_params, n_params + n_outs))
    devices = jax.devices()[:n_cores]
    mesh = Mesh(np.asarray(devices), ("core",))
    in_specs = (PartitionSpec("core"),) * (n_params + n_outs)
    out_specs = (PartitionSpec("core"),) * n_outs
    fn = jax.jit(
        shard_map(_body, mesh=mesh, in_specs=in_specs, out_specs=out_specs,
                  check_rep=False),
        donate_argnums=donate, keep_unused=True)

    def run(in_maps, iters=1):
        concat_in = [np.concatenate([np.asarray(in_maps[c][n]) for c in range(n_cores)], axis=0)
                     for n in in_names]
        dev_in = [jax.device_put(a) for a in concat_in]
        for a in dev_in:
            a.block_until_ready()
        times = []
        outs = None
        for _ in range(iters):
            zo = [np.concatenate([z] * n_cores, axis=0) for z in zero_outs]
            t0 = _time.perf_counter()
            outs = fn(*dev_in, *zo)
            for o in outs:
                o.block_until_ready()
            times.append(_time.perf_counter() - t0)
        results = []
        np_outs = [np.asarray(o) for o in outs]
        for c in range(n_cores):
            m = {}
            for i, nme in enumerate(out_names):
                per = np_outs[i].shape[0] // n_cores
                m[nme] = np_outs[i][c * per:(c + 1) * per]
            results.append(m)
        return results, times
    return run


def kernel(**inputs):
    cores, shared, dims, node_start = _prep(inputs)
    key = (dims["NW"], dims["WE"], dims["NWC"], dims["WNC"],
           dims["kposL"], dims["kposC"])
    if key not in _cache:
        nc = _build(dims)
        _cache[key] = (nc, make_runner(nc, NCORES))
    nc, run = _cache[key]

    in_maps = []
    for k in range(NCORES):
        m = dict(shared)
        m["xloc0"] = cores[k]["xloc0"]
        m["gnbr"] = cores[k]["gnbr"]
        m["shift"] = cores[k]["shift"]
        m["plnw"] = cores[k]["plnw"]
        m["cidx"] = cores[k]["cidx"]
        m["cshift"] = cores[k]["cshift"]
        m["cplnw"] = cores[k]["cplnw"]
        in_maps.append(m)
    res, times = run(in_maps, iters=int(os.environ.get('KERNEL_ITERS', '1')))
    kernel.last_times = times

    cpc = dims["cpc"]
    out = np.zeros((N_CRY, 64), np.float32)
    for k in range(NCORES):
        out[k * cpc:(k + 1) * cpc] = res[k]["out"][:cpc]
    return out

